# revision 1
# baseline (speedup 1.0000x reference)
"""Trainium2 Bass kernel for LoopConnectivityDecoder.

Math: out[i,j] (i<j) = sigmoid( sum_k W2[k] * relu(a'[i,k] + b'[k,j]) + b2 ),
symmetrized, zero diagonal; a' = X@W1[:,:32].T + b1, b' = (X@W1[:,32:].T).T.

Device strategy (8 cores, SPMD, per-core work fixed by host-side gathers):
- Signed scale folded into data: z_k = W2[k]*a' + W2[k]*b'. Then
  W2[k]*relu(a'+b') = max(z,0) if W2[k]>=0 else min(z,0).
- Upper triangle covered by 24 uniform (128 x 512) units, 3 per core.
- Per k: one K=4 bf16 matmul computes the outer sum z in PSUM at full fp32
  accuracy via hi/lo bf16 splitting: lhsT=[a_hi;a_lo;1;1], rhs=[1;1;b_hi;b_lo].
- k's are sign-grouped and chunked by 4 (groups zero-padded to 4-multiples):
  4 matmuls fill a (128,4,512) PSUM tile; ScalarE drains it with one fused
  relu (scale=+/-1 by sign) into SBUF; VectorE/GpSimd run 4-wide interleaved
  accumulate chains (scalar_tensor_tensor: acc = staged*(+/-1) + acc).
- Tail per unit: merge chains, sigmoid(+b2) on ScalarE, DMA out.
- Host scatters unit tiles into the full matrix, applies triu, mirrors.
"""

import numpy as np
import ml_dtypes

N = 1536
EMB = 32
H = 64
P = 128          # partition tile (rows per unit)
F = 512          # free-dim tile (cols per unit)
NCORES = 8
NBLK = N // P    # 12 row blocks
UNITS_PER_CORE = 3
CH = 4           # k's per chunk (PSUM tile = CH banks; build-time override)
LDG = 8          # k-slots per DMA load group

_cache = {}


def _unit_list():
    """24 (row_block, col0) units covering the upper-triangle staircase."""
    units = []
    for bi in range(NBLK):
        cols = N - P * bi
        nch = -(-cols // F)
        for t in range(nch):
            col0 = min(P * bi + F * t, N - F)
            units.append((bi, col0))
    assert len(units) == NCORES * UNITS_PER_CORE
    return units


def _slot_list(pos_mask, ch=CH):
    """Sign-grouped, zero-padded slot list.

    Returns (slots, chunk_signs): slots[i] is a k index or None (zero pad);
    chunk_signs[c] is +1/-1 for slots[ch*c : ch*(c+1)]."""
    pos = [k for k in range(H) if pos_mask[k]]
    neg = [k for k in range(H) if not pos_mask[k]]
    slots, signs = [], []
    for grp, sgn in ((pos, 1.0), (neg, -1.0)):
        if not grp:
            continue
        pad = (-len(grp)) % ch
        g = [None] * pad + grp
        slots += g
        signs += [sgn] * (len(g) // ch)
    assert len(slots) % ch == 0
    return slots, signs


def _build_module(pos_mask, repeat=1, n_dve_chunks=None, ablate="full",
                  stg_bufs=4, psum_bufs=2, stage_bf16=False, ch=CH):
    """Build + compile the Bass module. pos_mask: tuple of 64 bools."""
    from contextlib import ExitStack
    import concourse.tile as tile
    from concourse import bacc, mybir

    slots, signs = _slot_list(pos_mask, ch)
    S = len(slots)
    NCH = S // ch
    NLD = -(-S // LDG)
    if n_dve_chunks is None:
        n_dve_chunks = max(1, min(NCH - 1, round(NCH * 11 / 17)))
    if ablate == "nopool":
        n_dve_chunks = NCH

    nc = bacc.Bacc("TRN2", target_bir_lowering=False, debug=False,
                   num_devices=NCORES)
    A1_d = nc.dram_tensor("A1g", [4, S, UNITS_PER_CORE * P], mybir.dt.bfloat16,
                          kind="ExternalInput")
    B1_d = nc.dram_tensor("B1g", [4, S, UNITS_PER_CORE * F], mybir.dt.bfloat16,
                          kind="ExternalInput")
    b2_d = nc.dram_tensor("b2c", [P, 1], mybir.dt.float32, kind="ExternalInput")
    out_d = nc.dram_tensor("out", [UNITS_PER_CORE, P, F], mybir.dt.float32,
                           kind="ExternalOutput")

    with tile.TileContext(nc) as tc, ExitStack() as ctx:
        const = ctx.enter_context(tc.tile_pool(name="const", bufs=1))
        ld = ctx.enter_context(tc.tile_pool(name="ld", bufs=4))
        stg = ctx.enter_context(tc.tile_pool(name="stg", bufs=stg_bufs))
        accp = ctx.enter_context(tc.tile_pool(name="accp", bufs=2))
        outp = ctx.enter_context(tc.tile_pool(name="outp", bufs=2))
        psum = ctx.enter_context(tc.tile_pool(name="psum", bufs=psum_bufs, space="PSUM"))

        b2_t = const.tile([P, 1], mybir.dt.float32)
        nc.sync.dma_start(b2_t[:], b2_d[:])

        def body():
            for u in range(UNITS_PER_CORE):
                a_tiles, b_tiles = [], []
                for g in range(NLD):
                    s0 = g * LDG
                    sw = min(LDG, S - s0)
                    a_t = ld.tile([4, LDG, P], mybir.dt.bfloat16, tag="a")
                    nc.sync.dma_start(
                        a_t[:, 0:sw], A1_d[:, s0:s0 + sw, u * P:(u + 1) * P])
                    b_t = ld.tile([4, LDG, F], mybir.dt.bfloat16, tag="b")
                    nc.sync.dma_start(
                        b_t[:, 0:sw], B1_d[:, s0:s0 + sw, u * F:(u + 1) * F])
                    a_tiles.append(a_t)
                    b_tiles.append(b_t)

                accD = accN = None
                for c in range(NCH):
                    sgn = signs[c]
                    y = psum.tile([P, ch, F], mybir.dt.float32, tag="y")
                    for q in range(ch):
                        s = c * ch + q
                        g, off = s // LDG, s % LDG
                        nc.tensor.matmul(y[:, q],
                                         a_tiles[g][0:4, off, :],
                                         b_tiles[g][0:4, off, :],
                                         start=True, stop=True)
                    sdt = mybir.dt.bfloat16 if stage_bf16 else mybir.dt.float32
                    t4 = stg.tile([P, ch, F], sdt, tag="t4")
                    nc.scalar.activation(t4[:], y[:],
                                         mybir.ActivationFunctionType.Relu,
                                         scale=float(sgn))
                    if ablate == "noacc":
                        if c == NCH - 1:
                            accD = t4
                        continue
                    # accumulate: acc += sgn * t4 (4-wide interleaved chain)
                    on_dve = c < n_dve_chunks
                    if on_dve:
                        newacc = accp.tile([P, ch, F], mybir.dt.float32,
                                           tag="accD")
                        if accD is None:
                            nc.vector.tensor_scalar(newacc[:], t4[:],
                                                    float(sgn), None,
                                                    mybir.AluOpType.mult)
                        else:
                            nc.vector.scalar_tensor_tensor(
                                newacc[:], t4[:], float(sgn), accD[:],
                                mybir.AluOpType.mult, mybir.AluOpType.add)
                        accD = newacc
                    else:
                        # gpsimd: walrus rejects TensorScalarPtr on Pool, so
                        # chain with plain tensor_tensor add/subtract.
                        newacc = accp.tile([P, ch, F], mybir.dt.float32,
                                           tag="accN")
                        if accN is None:
                            accN = accp.tile([P, ch, F], mybir.dt.float32,
                                             tag="accN")
                            nc.gpsimd.memset(accN[:], 0.0)
                        op = (mybir.AluOpType.add if sgn > 0
                              else mybir.AluOpType.subtract)
                        nc.gpsimd.tensor_tensor(newacc[:], accN[:], t4[:], op)
                        accN = newacc

                # merge chains: logit = sum over ch slices (+ gpsimd chain)
                lg = outp.tile([P, F], mybir.dt.float32, tag="lg")
                def fold(eng, acc):
                    w = ch
                    while w > 1:
                        half = w // 2
                        nxt = outp.tile([P, half, F], mybir.dt.float32,
                                        tag=f"fold{half}")
                        eng.tensor_tensor(nxt[:], acc[:, 0:half],
                                          acc[:, half:2 * half],
                                          mybir.AluOpType.add)
                        acc, w = nxt, half
                    return acc
                aD = fold(nc.vector, accD)
                if accN is not None and ablate != "noacc":
                    aN = fold(nc.gpsimd, accN)
                    nc.vector.tensor_tensor(lg[:], aD[:, 0], aN[:, 0],
                                            mybir.AluOpType.add)
                else:
                    nc.vector.tensor_copy(lg[:], aD[:, 0])
                s_t = outp.tile([P, F], mybir.dt.float32, tag="s")
                nc.scalar.activation(s_t[:], lg[:],
                                     mybir.ActivationFunctionType.Sigmoid,
                                     bias=b2_t[:, 0:1], scale=1.0)
                nc.sync.dma_start(out_d[u], s_t[:])

        if repeat > 1:
            with tc.For_i(0, repeat, 1):
                body()
        else:
            body()

    nc.compile()
    return nc


def _split_bf16(x):
    """Split fp32 array into (hi, lo) bf16 arrays with hi+lo ~= x."""
    hi = x.astype(ml_dtypes.bfloat16)
    lo = (x - hi.astype(np.float32)).astype(ml_dtypes.bfloat16)
    return hi, lo


def _prep_inputs(loop_embeddings, W1, b1, W2, b2):
    X = np.asarray(loop_embeddings, dtype=np.float32)
    W1 = np.asarray(W1, dtype=np.float32)
    b1 = np.asarray(b1, dtype=np.float32)
    W2 = np.asarray(W2, dtype=np.float32)
    b2 = np.asarray(b2, dtype=np.float32)

    a = X @ W1[:, :EMB].T + b1          # (N, H)
    bm = X @ W1[:, EMB:].T              # (N, H)
    w2 = W2[0]

    az = (w2[None, :] * a).T            # (H, N): z-contribution rows (i)
    bz = (w2[None, :] * bm).T           # (H, N): z-contribution rows (j)
    az_hi, az_lo = _split_bf16(az)
    bz_hi, bz_lo = _split_bf16(bz)

    pos_mask = tuple(bool(v) for v in (w2 >= 0))
    slots, _ = _slot_list(pos_mask)
    S = len(slots)
    units = _unit_list()

    in_maps = []
    for core in range(NCORES):
        A1g = np.zeros((4, S, UNITS_PER_CORE * P), dtype=ml_dtypes.bfloat16)
        B1g = np.zeros((4, S, UNITS_PER_CORE * F), dtype=ml_dtypes.bfloat16)
        for u in range(UNITS_PER_CORE):
            bi, col0 = units[core * UNITS_PER_CORE + u]
            for s, k in enumerate(slots):
                if k is None:
                    continue
                A1g[0, s, u * P:(u + 1) * P] = az_hi[k, bi * P:(bi + 1) * P]
                A1g[1, s, u * P:(u + 1) * P] = az_lo[k, bi * P:(bi + 1) * P]
                A1g[2, s, u * P:(u + 1) * P] = 1.0
                A1g[3, s, u * P:(u + 1) * P] = 1.0
                B1g[0, s, u * F:(u + 1) * F] = 1.0
                B1g[1, s, u * F:(u + 1) * F] = 1.0
                B1g[2, s, u * F:(u + 1) * F] = bz_hi[k, col0:col0 + F]
                B1g[3, s, u * F:(u + 1) * F] = bz_lo[k, col0:col0 + F]
        in_maps.append({
            "A1g": A1g,
            "B1g": B1g,
            "b2c": np.full((P, 1), b2[0], dtype=np.float32),
        })
    return in_maps, pos_mask, units


def kernel(loop_embeddings, W1, b1, W2, b2):
    from concourse.bass_utils import run_bass_kernel_spmd

    in_maps, pos_mask, units = _prep_inputs(loop_embeddings, W1, b1, W2, b2)

    if pos_mask not in _cache:
        _cache[pos_mask] = _build_module(pos_mask)
    nc = _cache[pos_mask]

    res = run_bass_kernel_spmd(nc, in_maps, list(range(NCORES)))

    s = np.zeros((N, N), dtype=np.float32)
    for core in range(NCORES):
        o = res.results[core]["out"]
        for u in range(UNITS_PER_CORE):
            bi, col0 = units[core * UNITS_PER_CORE + u]
            s[bi * P:(bi + 1) * P, col0:col0 + F] = o[u]
    up = np.triu(s, 1)
    return (up + up.T).astype(np.float32)



# revision 8
# speedup vs baseline: 3.2306x; 3.2306x over previous
"""Trainium2 Bass kernel for LoopConnectivityDecoder (wire-optimized).

Math: out[i,j] (i<j) = sigmoid( sum_k W2[k] * relu(a'[i,k] + b'[k,j]) + b2 ),
symmetrized, zero diagonal; a' = X@W1[:,:32].T + b1, b' = (X@W1[:,32:].T).T.

The device work (~0.2ms) is dwarfed by the axon tunnel (~60ms RTT,
~58MB/s H2D, ~27MB/s D2H), so the design minimizes wire bytes and
per-call dispatch overhead:

- Signed scale folded into data host-side: az = (W2*a').T, bz = (W2*b').T,
  shipped as a single fp16 plane each (rel err ~1e-4, gate is 2e-2).
- Upper triangle covered by 24 uniform (128 x 512) units, 3 per core.
- Per slot k: one K=2 fp16 matmul computes the outer sum z = az[i]+bz[j]
  in PSUM: lhsT=[az_k;1], rhs=[1;bz_k]; the ones planes are memset on
  device, never shipped.
- k's sign-grouped and chunked by 4: 4 matmuls fill a (128,4,512) PSUM
  tile; ScalarE drains with fused relu (scale=+/-1); VectorE/GpSimd run
  4-wide interleaved accumulate chains.
- Tail per unit: merge chains, sigmoid(+b2), quantize to uint8
  (round(255*s), ~2e-3 rel err) to halve the D2H bytes, DMA out.
- The jitted shard_map dispatcher is cached across calls (the stock
  run_bass_kernel_spmd re-traces and re-lowers every call).
- Host scatters unit tiles, zeroes the 12 diagonal 128x128 lower
  triangles, mirrors.
"""

import numpy as np

N = 1536
EMB = 32
H = 64
P = 128          # partition tile (rows per unit)
F = 512          # free-dim tile (cols per unit)
NCORES = 8
NBLK = N // P    # 12 row blocks
UNITS_PER_CORE = 3
CH = 4           # k's per chunk (PSUM tile = CH banks)

_cache = {}


def _unit_list():
    """24 (row_block, col0) units covering the upper-triangle staircase."""
    units = []
    for bi in range(NBLK):
        cols = N - P * bi
        nch = -(-cols // F)
        for t in range(nch):
            col0 = min(P * bi + F * t, N - F)
            units.append((bi, col0))
    assert len(units) == NCORES * UNITS_PER_CORE
    return units


def _slot_list(pos_mask, ch=CH):
    """Sign-grouped, zero-padded slot list.

    Returns (slots, chunk_signs): slots[i] is a k index or None (zero pad);
    chunk_signs[c] is +1/-1 for slots[ch*c : ch*(c+1)]."""
    pos = [k for k in range(H) if pos_mask[k]]
    neg = [k for k in range(H) if not pos_mask[k]]
    slots, signs = [], []
    for grp, sgn in ((pos, 1.0), (neg, -1.0)):
        if not grp:
            continue
        pad = (-len(grp)) % ch
        g = [None] * pad + grp
        slots += g
        signs += [sgn] * (len(g) // ch)
    assert len(slots) % ch == 0
    return slots, signs


def _build_module(pos_mask):
    """Build + compile the Bass module. pos_mask: tuple of 64 bools."""
    from contextlib import ExitStack
    import concourse.tile as tile
    from concourse import bacc, mybir

    slots, signs = _slot_list(pos_mask)
    S = len(slots)
    NCH = S // CH
    n_dve_chunks = max(1, min(NCH - 1, round(NCH * 11 / 17)))
    AOFF = 0
    BOFF = UNITS_PER_CORE * P  # 384

    nc = bacc.Bacc("TRN2", target_bir_lowering=False, debug=False,
                   num_devices=NCORES)
    AB_d = nc.dram_tensor("ab", [1, S, UNITS_PER_CORE * (P + F)],
                          mybir.dt.float16, kind="ExternalInput")
    b2_d = nc.dram_tensor("b2c", [P, 1], mybir.dt.float32,
                          kind="ExternalInput")
    out_d = nc.dram_tensor("out", [UNITS_PER_CORE, P, F], mybir.dt.uint8,
                           kind="ExternalOutput")

    with tile.TileContext(nc) as tc, ExitStack() as ctx:
        const = ctx.enter_context(tc.tile_pool(name="const", bufs=1))
        stg = ctx.enter_context(tc.tile_pool(name="stg", bufs=4))
        accp = ctx.enter_context(tc.tile_pool(name="accp", bufs=2))
        outp = ctx.enter_context(tc.tile_pool(name="outp", bufs=2))
        psum = ctx.enter_context(tc.tile_pool(name="psum", bufs=2,
                                              space="PSUM"))

        b2_t = const.tile([P, 1], mybir.dt.float32, tag="b2")
        nc.sync.dma_start(b2_t[:], b2_d[:])

        # Persistent load tiles: plane layout for the K=2 outer-sum matmul
        # lhsT=[az;1], rhs=[1;bz]. Ones planes are memset once; per-unit
        # DMAs overwrite only the data planes (Tile tracks the WAR deps).
        a_t = const.tile([2, S, P], mybir.dt.float16, tag="a")
        b_t = const.tile([2, S, F], mybir.dt.float16, tag="b")
        # compute-engine APs must start at partition 0: memset the full
        # 2-partition tiles to 1.0; per-unit DMAs overwrite the data plane.
        nc.gpsimd.memset(a_t[:], 1.0)
        nc.gpsimd.memset(b_t[:], 1.0)

        for u in range(UNITS_PER_CORE):
            nc.sync.dma_start(a_t[0:1],
                              AB_d[:, :, AOFF + u * P:AOFF + (u + 1) * P])
            nc.sync.dma_start(b_t[1:2],
                              AB_d[:, :, BOFF + u * F:BOFF + (u + 1) * F])

            accD = accN = None
            for c in range(NCH):
                sgn = signs[c]
                y = psum.tile([P, CH, F], mybir.dt.float32, tag="y")
                for q in range(CH):
                    s = c * CH + q
                    nc.tensor.matmul(y[:, q], a_t[0:2, s, :], b_t[0:2, s, :],
                                     start=True, stop=True)
                t4 = stg.tile([P, CH, F], mybir.dt.float32, tag="t4")
                nc.scalar.activation(t4[:], y[:],
                                     mybir.ActivationFunctionType.Relu,
                                     scale=float(sgn))
                # accumulate: acc += sgn * t4 (4-wide interleaved chain)
                if c < n_dve_chunks:
                    newacc = accp.tile([P, CH, F], mybir.dt.float32,
                                       tag="accD")
                    if accD is None:
                        nc.vector.tensor_scalar(newacc[:], t4[:], float(sgn),
                                                None, mybir.AluOpType.mult)
                    else:
                        nc.vector.scalar_tensor_tensor(
                            newacc[:], t4[:], float(sgn), accD[:],
                            mybir.AluOpType.mult, mybir.AluOpType.add)
                    accD = newacc
                else:
                    # gpsimd: walrus rejects TensorScalarPtr on Pool, so
                    # chain with plain tensor_tensor add/subtract.
                    newacc = accp.tile([P, CH, F], mybir.dt.float32,
                                       tag="accN")
                    if accN is None:
                        accN = accp.tile([P, CH, F], mybir.dt.float32,
                                         tag="accN")
                        nc.gpsimd.memset(accN[:], 0.0)
                    op = (mybir.AluOpType.add if sgn > 0
                          else mybir.AluOpType.subtract)
                    nc.gpsimd.tensor_tensor(newacc[:], accN[:], t4[:], op)
                    accN = newacc

            # merge chains: logit = sum over CH slices (+ gpsimd chain)
            lg = outp.tile([P, F], mybir.dt.float32, tag="lg")

            def fold(eng, acc):
                w = CH
                while w > 1:
                    half = w // 2
                    nxt = outp.tile([P, half, F], mybir.dt.float32,
                                    tag=f"fold{half}")
                    eng.tensor_tensor(nxt[:], acc[:, 0:half],
                                      acc[:, half:2 * half],
                                      mybir.AluOpType.add)
                    acc, w = nxt, half
                return acc

            aD = fold(nc.vector, accD)
            if accN is not None:
                aN = fold(nc.gpsimd, accN)
                nc.vector.tensor_tensor(lg[:], aD[:, 0], aN[:, 0],
                                        mybir.AluOpType.add)
            else:
                nc.vector.tensor_copy(lg[:], aD[:, 0])
            s_t = outp.tile([P, F], mybir.dt.float32, tag="s")
            nc.scalar.activation(s_t[:], lg[:],
                                 mybir.ActivationFunctionType.Sigmoid,
                                 bias=b2_t[:, 0:1], scale=1.0)
            # quantize: uint8 round(255*s) halves the D2H bytes
            q_t = outp.tile([P, F], mybir.dt.uint8, tag="q")
            nc.scalar.activation(q_t[:], s_t[:],
                                 mybir.ActivationFunctionType.Copy,
                                 bias=0.49, scale=255.0)
            nc.sync.dma_start(out_d[u], q_t[:])

    nc.compile()
    return nc


def _build_runner(nc):
    """Cached jitted shard_map dispatcher for an SPMD Bass module."""
    import jax
    from concourse import bass2jax, mybir
    from jax.experimental.shard_map import shard_map
    from jax.sharding import Mesh, PartitionSpec

    bass2jax.install_neuronx_cc_hook()
    assert nc.dbg_addr is None
    partition_name = (nc.partition_id_tensor.name
                      if nc.partition_id_tensor else None)
    in_names, out_names, out_avals = [], [], []
    for alloc in nc.m.functions[0].allocations:
        if not isinstance(alloc, mybir.MemoryLocationSet):
            continue
        name = alloc.memorylocations[0].name
        if alloc.kind == "ExternalInput":
            if name != partition_name:
                in_names.append(name)
        elif alloc.kind == "ExternalOutput":
            out_names.append(name)
            out_avals.append(jax.core.ShapedArray(
                tuple(alloc.tensor_shape), mybir.dt.np(alloc.dtype)))
    n_params = len(in_names)
    n_outs = len(out_names)
    all_names = tuple(in_names + out_names
                      + ([partition_name] if partition_name else []))
    donate = tuple(range(n_params, n_params + n_outs))

    def _body(*args):
        operands = list(args)
        if partition_name is not None:
            operands.append(bass2jax.partition_id_tensor())
        outs = bass2jax._bass_exec_p.bind(
            *operands,
            out_avals=tuple(out_avals),
            in_names=all_names,
            out_names=tuple(out_names),
            lowering_input_output_aliases=(),
            sim_require_finite=True,
            sim_require_nnan=True,
            nc=nc,
        )
        return tuple(outs)

    devices = jax.devices()[:NCORES]
    assert len(devices) == NCORES
    mesh = Mesh(np.asarray(devices), ("core",))
    sharded = jax.jit(
        shard_map(_body, mesh=mesh,
                  in_specs=(PartitionSpec("core"),) * (n_params + n_outs),
                  out_specs=(PartitionSpec("core"),) * n_outs,
                  check_rep=False),
        donate_argnums=donate, keep_unused=True)
    return sharded


_UNITS = _unit_list()
_DIAG_MASK = np.triu(np.ones((P, P), dtype=np.float32), k=1)


def kernel(loop_embeddings, W1, b1, W2, b2):
    X = np.asarray(loop_embeddings, dtype=np.float32)
    W1 = np.asarray(W1, dtype=np.float32)
    b1 = np.asarray(b1, dtype=np.float32)
    W2 = np.asarray(W2, dtype=np.float32)
    b2 = np.asarray(b2, dtype=np.float32)

    a = X @ W1[:, :EMB].T + b1          # (N, H)
    bm = X @ W1[:, EMB:].T              # (N, H)
    w2 = W2[0]
    az = np.ascontiguousarray((w2[None, :] * a).T, dtype=np.float32)
    bz = np.ascontiguousarray((w2[None, :] * bm).T, dtype=np.float32)

    pos_mask = tuple(bool(v) for v in (w2 >= 0))
    ent = _cache.get(pos_mask)
    if ent is None:
        nc = _build_module(pos_mask)
        ent = {"nc": nc, "runner": _build_runner(nc),
               "slots": _slot_list(pos_mask)[0]}
        _cache[pos_mask] = ent
    slots = ent["slots"]
    S = len(slots)

    # slot-expanded az/bz in fp16, zero rows for pad slots
    kidx = np.array([0 if k is None else k for k in slots], dtype=np.int64)
    valid = np.array([k is not None for k in slots], dtype=bool)
    az_s = az[kidx].astype(np.float16)
    bz_s = bz[kidx].astype(np.float16)
    az_s[~valid] = 0
    bz_s[~valid] = 0

    # pack per-core input: [S, 3*128 | 3*512] fp16
    AB = np.empty((NCORES, S, UNITS_PER_CORE * (P + F)), dtype=np.float16)
    BOFF = UNITS_PER_CORE * P
    for g, (bi, col0) in enumerate(_UNITS):
        core, u = divmod(g, UNITS_PER_CORE)
        AB[core, :, u * P:(u + 1) * P] = az_s[:, bi * P:(bi + 1) * P]
        AB[core, :, BOFF + u * F:BOFF + (u + 1) * F] = bz_s[:, col0:col0 + F]

    args = [
        AB,  # (NCORES, S, 1920): per-core BIR shape is [1, S, 1920]
        np.full((NCORES * P, 1), b2[0], dtype=np.float32),
        np.zeros((NCORES * UNITS_PER_CORE, P, F), dtype=np.uint8),
    ]
    out_arrs = ent["runner"](*args)
    q = np.asarray(out_arrs[0])          # (24, P, F) uint8

    tiles = q.astype(np.float32)
    tiles *= np.float32(1.0 / 255.0)
    s = np.zeros((N, N), dtype=np.float32)
    for g, (bi, col0) in enumerate(_UNITS):
        s[bi * P:(bi + 1) * P, col0:col0 + F] = tiles[g]
    # zero below-diagonal writes: clamped units (col0 < bi*P) overhang
    # left of the diagonal block, and the diagonal 128x128 blocks hold
    # their lower triangle.
    for bi, col0 in _UNITS:
        if col0 < bi * P:
            s[bi * P:(bi + 1) * P, col0:bi * P] = 0.0
    for bi in range(NBLK):
        s[bi * P:(bi + 1) * P, bi * P:(bi + 1) * P] *= _DIAG_MASK
    return s + s.T


# revision 13
# speedup vs baseline: 4.0734x; 1.2609x over previous
"""Trainium2 Bass kernel for LoopConnectivityDecoder (wire-optimized).

Math: out[i,j] (i<j) = sigmoid( sum_k W2[k] * relu(a'[i,k] + b'[k,j]) + b2 ),
symmetrized, zero diagonal; a' = X@W1[:,:32].T + b1, b' = (X@W1[:,32:].T).T.

The device work (~0.2ms) is dwarfed by the axon tunnel (~60ms RTT,
~58MB/s H2D, ~27MB/s D2H), so the design minimizes wire bytes and
per-call dispatch overhead:

- Signed scale folded into data host-side: az = (W2*a').T, bz = (W2*b').T,
  shipped as a single fp16 plane each (rel err ~1e-4, gate is 2e-2).
- Upper triangle covered by 24 uniform (128 x 512) units, 3 per core.
- Per slot k: one K=2 fp16 matmul computes the outer sum z = az[i]+bz[j]
  in PSUM: lhsT=[az_k;1], rhs=[1;bz_k]; the ones planes are memset on
  device, never shipped.
- k's sign-grouped and chunked by 4: 4 matmuls fill a (128,4,512) PSUM
  tile; ScalarE drains with fused relu (scale=+/-1); VectorE/GpSimd run
  4-wide interleaved accumulate chains.
- Tail per unit: merge chains, sigmoid(+b2), quantize to uint8
  (round(255*s), ~2e-3 rel err) to halve the D2H bytes, DMA out.
- The jitted shard_map dispatcher is cached across calls (the stock
  run_bass_kernel_spmd re-traces and re-lowers every call).
- Host scatters unit tiles, zeroes the 12 diagonal 128x128 lower
  triangles, mirrors.
"""

import numpy as np

N = 1536
EMB = 32
H = 64
P = 128          # partition tile (rows per unit)
F = 512          # free-dim tile (cols per unit)
NCORES = 8
NBLK = N // P    # 12 row blocks
UNITS_PER_CORE = 3
CH = 4           # k's per chunk (PSUM tile = CH banks)

_cache = {}


def _unit_list():
    """24 (row_block, col0) units covering the upper-triangle staircase."""
    units = []
    for bi in range(NBLK):
        cols = N - P * bi
        nch = -(-cols // F)
        for t in range(nch):
            col0 = min(P * bi + F * t, N - F)
            units.append((bi, col0))
    assert len(units) == NCORES * UNITS_PER_CORE
    return units


def _slot_list(pos_mask, ch=CH):
    """Sign-grouped, zero-padded slot list.

    Returns (slots, chunk_signs): slots[i] is a k index or None (zero pad);
    chunk_signs[c] is +1/-1 for slots[ch*c : ch*(c+1)]."""
    pos = [k for k in range(H) if pos_mask[k]]
    neg = [k for k in range(H) if not pos_mask[k]]
    slots, signs = [], []
    for grp, sgn in ((pos, 1.0), (neg, -1.0)):
        if not grp:
            continue
        pad = (-len(grp)) % ch
        g = [None] * pad + grp
        slots += g
        signs += [sgn] * (len(g) // ch)
    assert len(slots) % ch == 0
    return slots, signs


def _build_module(pos_mask):
    """Build + compile the Bass module. pos_mask: tuple of 64 bools."""
    from contextlib import ExitStack
    import concourse.tile as tile
    from concourse import bacc, mybir

    slots, signs = _slot_list(pos_mask)
    S = len(slots)
    NCH = S // CH
    n_dve_chunks = max(1, min(NCH - 1, round(NCH * 11 / 17)))
    AOFF = 0
    BOFF = UNITS_PER_CORE * P  # 384

    nc = bacc.Bacc("TRN2", target_bir_lowering=False, debug=False,
                   num_devices=NCORES)
    AB_d = nc.dram_tensor("ab", [1, S, UNITS_PER_CORE * (P + F)],
                          mybir.dt.float16, kind="ExternalInput")
    b2_d = nc.dram_tensor("b2c", [P, 1], mybir.dt.float32,
                          kind="ExternalInput")
    out_d = nc.dram_tensor("out", [UNITS_PER_CORE, P, F], mybir.dt.uint8,
                           kind="ExternalOutput")

    with tile.TileContext(nc) as tc, ExitStack() as ctx:
        const = ctx.enter_context(tc.tile_pool(name="const", bufs=1))
        stg = ctx.enter_context(tc.tile_pool(name="stg", bufs=4))
        accp = ctx.enter_context(tc.tile_pool(name="accp", bufs=2))
        outp = ctx.enter_context(tc.tile_pool(name="outp", bufs=2))
        psum = ctx.enter_context(tc.tile_pool(name="psum", bufs=2,
                                              space="PSUM"))

        b2_t = const.tile([P, 1], mybir.dt.float32, tag="b2")
        nc.sync.dma_start(b2_t[:], b2_d[:])

        # Persistent load tiles: plane layout for the K=2 outer-sum matmul
        # lhsT=[az;1], rhs=[1;bz]. Ones planes are memset once; per-unit
        # DMAs overwrite only the data planes (Tile tracks the WAR deps).
        a_t = const.tile([2, S, P], mybir.dt.float16, tag="a")
        b_t = const.tile([2, S, F], mybir.dt.float16, tag="b")
        # compute-engine APs must start at partition 0: memset the full
        # 2-partition tiles to 1.0; per-unit DMAs overwrite the data plane.
        nc.gpsimd.memset(a_t[:], 1.0)
        nc.gpsimd.memset(b_t[:], 1.0)

        for u in range(UNITS_PER_CORE):
            nc.sync.dma_start(a_t[0:1],
                              AB_d[:, :, AOFF + u * P:AOFF + (u + 1) * P])
            nc.sync.dma_start(b_t[1:2],
                              AB_d[:, :, BOFF + u * F:BOFF + (u + 1) * F])

            accD = accN = None
            for c in range(NCH):
                sgn = signs[c]
                y = psum.tile([P, CH, F], mybir.dt.float32, tag="y")
                for q in range(CH):
                    s = c * CH + q
                    nc.tensor.matmul(y[:, q], a_t[0:2, s, :], b_t[0:2, s, :],
                                     start=True, stop=True)
                t4 = stg.tile([P, CH, F], mybir.dt.float32, tag="t4")
                nc.scalar.activation(t4[:], y[:],
                                     mybir.ActivationFunctionType.Relu,
                                     scale=float(sgn))
                # accumulate: acc += sgn * t4 (4-wide interleaved chain)
                if c < n_dve_chunks:
                    newacc = accp.tile([P, CH, F], mybir.dt.float32,
                                       tag="accD")
                    if accD is None:
                        nc.vector.tensor_scalar(newacc[:], t4[:], float(sgn),
                                                None, mybir.AluOpType.mult)
                    else:
                        nc.vector.scalar_tensor_tensor(
                            newacc[:], t4[:], float(sgn), accD[:],
                            mybir.AluOpType.mult, mybir.AluOpType.add)
                    accD = newacc
                else:
                    # gpsimd: walrus rejects TensorScalarPtr on Pool, so
                    # chain with plain tensor_tensor add/subtract.
                    newacc = accp.tile([P, CH, F], mybir.dt.float32,
                                       tag="accN")
                    if accN is None:
                        accN = accp.tile([P, CH, F], mybir.dt.float32,
                                         tag="accN")
                        nc.gpsimd.memset(accN[:], 0.0)
                    op = (mybir.AluOpType.add if sgn > 0
                          else mybir.AluOpType.subtract)
                    nc.gpsimd.tensor_tensor(newacc[:], accN[:], t4[:], op)
                    accN = newacc

            # merge chains: logit = sum over CH slices (+ gpsimd chain)
            lg = outp.tile([P, F], mybir.dt.float32, tag="lg")

            def fold(eng, acc):
                w = CH
                while w > 1:
                    half = w // 2
                    nxt = outp.tile([P, half, F], mybir.dt.float32,
                                    tag=f"fold{half}")
                    eng.tensor_tensor(nxt[:], acc[:, 0:half],
                                      acc[:, half:2 * half],
                                      mybir.AluOpType.add)
                    acc, w = nxt, half
                return acc

            aD = fold(nc.vector, accD)
            if accN is not None:
                aN = fold(nc.gpsimd, accN)
                nc.vector.tensor_tensor(lg[:], aD[:, 0], aN[:, 0],
                                        mybir.AluOpType.add)
            else:
                nc.vector.tensor_copy(lg[:], aD[:, 0])
            s_t = outp.tile([P, F], mybir.dt.float32, tag="s")
            nc.scalar.activation(s_t[:], lg[:],
                                 mybir.ActivationFunctionType.Sigmoid,
                                 bias=b2_t[:, 0:1], scale=1.0)
            # quantize: uint8 round(255*s) halves the D2H bytes
            q_t = outp.tile([P, F], mybir.dt.uint8, tag="q")
            nc.scalar.activation(q_t[:], s_t[:],
                                 mybir.ActivationFunctionType.Copy,
                                 bias=0.49, scale=255.0)
            nc.sync.dma_start(out_d[u], q_t[:])

    nc.compile()
    return nc


def _build_runner(nc):
    """Cached jitted shard_map dispatcher for an SPMD Bass module."""
    import jax
    from concourse import bass2jax, mybir
    from jax.experimental.shard_map import shard_map
    from jax.sharding import Mesh, PartitionSpec

    bass2jax.install_neuronx_cc_hook()
    assert nc.dbg_addr is None
    partition_name = (nc.partition_id_tensor.name
                      if nc.partition_id_tensor else None)
    in_names, out_names, out_avals = [], [], []
    for alloc in nc.m.functions[0].allocations:
        if not isinstance(alloc, mybir.MemoryLocationSet):
            continue
        name = alloc.memorylocations[0].name
        if alloc.kind == "ExternalInput":
            if name != partition_name:
                in_names.append(name)
        elif alloc.kind == "ExternalOutput":
            out_names.append(name)
            out_avals.append(jax.core.ShapedArray(
                tuple(alloc.tensor_shape), mybir.dt.np(alloc.dtype)))
    n_params = len(in_names)
    n_outs = len(out_names)
    all_names = tuple(in_names + out_names
                      + ([partition_name] if partition_name else []))
    donate = tuple(range(n_params, n_params + n_outs))

    def _body(*args):
        operands = list(args)
        if partition_name is not None:
            operands.append(bass2jax.partition_id_tensor())
        outs = bass2jax._bass_exec_p.bind(
            *operands,
            out_avals=tuple(out_avals),
            in_names=all_names,
            out_names=tuple(out_names),
            lowering_input_output_aliases=(),
            sim_require_finite=True,
            sim_require_nnan=True,
            nc=nc,
        )
        return tuple(outs)

    devices = jax.devices()[:NCORES]
    assert len(devices) == NCORES
    mesh = Mesh(np.asarray(devices), ("core",))
    sharded = jax.jit(
        shard_map(_body, mesh=mesh,
                  in_specs=(PartitionSpec("core"),) * (n_params + n_outs),
                  out_specs=(PartitionSpec("core"),) * n_outs,
                  check_rep=False),
        donate_argnums=donate, keep_unused=True)
    return sharded


_UNITS = _unit_list()

# Per-unit fused dequant+ownership mask: 1/255 where this tile is the
# unique owner of a strictly-upper cell (clamped units overlap their
# predecessor in a row block; the predecessor owns the overlap), else 0.
# With unique ownership, assembly can += both the tile and its transpose
# in any order.
_TILE_MASK = []
_covered = {}
for _bi, _col0 in _UNITS:
    _start = max(_col0, _covered.get(_bi, 0))
    _ii = _bi * P + np.arange(P)[:, None]
    _jj = _col0 + np.arange(F)[None, :]
    _m = ((_jj > _ii) & (_jj >= _start)).astype(np.float32)
    _m *= np.float32(1.0 / 255.0)
    _covered[_bi] = _col0 + F
    _TILE_MASK.append(_m)


def _get_zeros_maker():
    import jax
    import jax.numpy as jnp
    from jax.sharding import Mesh, NamedSharding, PartitionSpec

    mesh = Mesh(np.asarray(jax.devices()[:NCORES]), ("core",))
    sh = NamedSharding(mesh, PartitionSpec("core"))
    return jax.jit(
        lambda: jnp.zeros((NCORES * UNITS_PER_CORE, P, F), jnp.uint8),
        out_shardings=sh)


def kernel(loop_embeddings, W1, b1, W2, b2):
    X = np.asarray(loop_embeddings, dtype=np.float32)
    W1 = np.asarray(W1, dtype=np.float32)
    b1 = np.asarray(b1, dtype=np.float32)
    W2 = np.asarray(W2, dtype=np.float32)
    b2 = np.asarray(b2, dtype=np.float32)

    a = X @ W1[:, :EMB].T + b1          # (N, H)
    bm = X @ W1[:, EMB:].T              # (N, H)
    w2 = W2[0]
    az = np.ascontiguousarray((w2[None, :] * a).T, dtype=np.float32)
    bz = np.ascontiguousarray((w2[None, :] * bm).T, dtype=np.float32)

    pos_mask = tuple(bool(v) for v in (w2 >= 0))
    ent = _cache.get(pos_mask)
    if ent is None:
        nc = _build_module(pos_mask)
        ent = {"nc": nc, "runner": _build_runner(nc),
               "slots": _slot_list(pos_mask)[0],
               "zeros_maker": _get_zeros_maker()}
        _cache[pos_mask] = ent
    slots = ent["slots"]
    S = len(slots)
    # kick off the on-device zero-fill of the donated output buffers now;
    # it runs remotely while the host packs AB.
    zeros_fut = ent["zeros_maker"]()

    # slot-expanded az/bz in fp16, zero rows for pad slots
    kidx = np.array([0 if k is None else k for k in slots], dtype=np.int64)
    valid = np.array([k is not None for k in slots], dtype=bool)
    az_s = az[kidx].astype(np.float16)
    bz_s = bz[kidx].astype(np.float16)
    az_s[~valid] = 0
    bz_s[~valid] = 0

    # pack per-core input: [S, 3*128 | 3*512] fp16
    AB = np.empty((NCORES, S, UNITS_PER_CORE * (P + F)), dtype=np.float16)
    BOFF = UNITS_PER_CORE * P
    for g, (bi, col0) in enumerate(_UNITS):
        core, u = divmod(g, UNITS_PER_CORE)
        AB[core, :, u * P:(u + 1) * P] = az_s[:, bi * P:(bi + 1) * P]
        AB[core, :, BOFF + u * F:BOFF + (u + 1) * F] = bz_s[:, col0:col0 + F]

    args = [
        AB,  # (NCORES, S, 1920): per-core BIR shape is [1, S, 1920]
        np.full((NCORES * P, 1), b2[0], dtype=np.float32),
        zeros_fut,
    ]
    out_arrs = ent["runner"](*args)
    q = np.asarray(out_arrs[0])          # (24, P, F) uint8

    # fused dequant+ownership-mask, accumulated into both triangles
    r = np.zeros((N, N), dtype=np.float32)
    for g, (bi, col0) in enumerate(_UNITS):
        t = q[g] * _TILE_MASK[g]
        r[bi * P:(bi + 1) * P, col0:col0 + F] += t
        r[col0:col0 + F, bi * P:(bi + 1) * P] += t.T
    return r
